# revision 7
# baseline (speedup 1.0000x reference)
"""Distributed multi-head attention kernel for 8 Trainium2 NeuronCores.

Problem: B=2, S=2048, HIDDEN=1024, 16 heads x 64 dims, causal softmax,
torch-Linear style projections (x @ W.T + b), fp32.

Sharding (tensor parallel over heads x data parallel over batch):
  core c = b*4 + g handles batch b and head-group g (4 heads, 256 dims).
  Each core computes q/k/v projections for its head group, causal
  attention, and a partial output projection A_g @ Wo_g.T -> [S, 1024].
  The host sums the 4 partials per batch (the TP all-reduce) and adds
  the bias terms (bo and bv @ Wo.T, which commute through softmax
  because softmax rows sum to 1).

Device-side layout choices:
  - q/k are produced transposed ([head_dim, seq]) so attention scores
    can be computed directly as S^T[k_pos, q_pos] tiles with the head
    dim as the matmul contraction; two heads of a pair are packed into
    one 128-partition tile (partitions 0-63 / 64-127) and their score
    matmuls run concurrently in disjoint PE row groups.
  - softmax skips the max-subtraction (scores/8 are O(+-3) for this
    problem, exp cannot overflow) so exp(S^T/8) is a single activation
    pass per tile and row sums come for free from an extra ones column
    appended to v in the P@V matmul (output row 64 = sum over k of P).
  - causal masking: whole invalid 128-col blocks are simply not
    computed (matmul free-dim offset); the diagonal 128x128 block gets
    -1e9 added via one tiny bf16 matmul (strict-upper-triangular
    constant against identity) before the exp.
  - normalization 1/rowsum is done per 512-wide q chunk: DVE
    reciprocal on the sums row, GPSIMD partition-broadcast to 64
    partitions, one DVE multiply into A.
  - all large matmuls use float32r (full fp32 data, fast PE mode).
"""

import numpy as np
import ml_dtypes

import concourse.bass as bass
import concourse.mybir as mybir
import concourse.tile as tile
from concourse import bass_utils

S = 2048
HID = 1024
D = 64
B = 2
NCORES = 8
HPC = 4            # heads per core
GD = HPC * D       # 256 dims per head group
KT = HID // 128    # 8 hidden k-tiles
NT = S // 128      # 16 seq tiles
NSUP = S // 512    # 4 q supertiles

f32 = mybir.dt.float32
f32r = mybir.dt.float32r
bf16 = mybir.dt.bfloat16
AF = mybir.ActivationFunctionType

TRACE = False
TRACE_KW = {}


def _r(ap):
    return ap


def _split_sem_waits(nc, limit=1):
    """walrus in this container accepts only one semaphore wait per
    instruction; hoist excess waits into wait-only EventSemaphore
    instructions inserted just before the offender on the same engine."""
    counter = 0
    for fn in nc.m.functions:
        for blk in fn.blocks:
            out = []
            for inst in blk.instructions:
                si = getattr(inst, "sync_info", None)
                if si is not None:
                    waits = list(si.on_wait)
                    sem_w = [w for w in waits if w.sync_type == "semaphore"]
                    other = [w for w in waits if w.sync_type != "semaphore"]
                    if len(sem_w) > limit:
                        excess, keep = sem_w[:-limit], sem_w[-limit:]
                        for i in range(0, len(excess), limit):
                            counter += 1
                            ev = mybir.InstEventSemaphore(
                                name=f"WSPLIT-{counter}",
                                ins=[],
                                outs=[],
                                sync_info=mybir.SyncInfo(
                                    on_wait=excess[i : i + limit], on_update=[]
                                ),
                            )
                            ev.engine = inst.engine
                            out.append(ev)
                        inst.sync_info = mybir.SyncInfo(
                            on_wait=other + keep, on_update=list(si.on_update)
                        )
                out.append(inst)
            blk.instructions = out


def _build_body(nc, tc, dram, out):
    import contextlib

    with contextlib.ExitStack() as ctx:
        # ---- persistent tiles (bufs=1, one slot per tag) ----
        pers = ctx.enter_context(tc.tile_pool(name="pers", bufs=1))

        def ptile(shape, dtype, name):
            return pers.tile(shape, dtype, name=name, tag=name)

        wo_sb = ptile([64, HPC * HID], f32r, "wo_sb")
        bq_sb = ptile([128, 2], f32, "bq_sb")
        bk_sb = ptile([128, 2], f32, "bk_sb")
        tri_sb = ptile([128, 128], bf16, "tri_sb")
        id_sb = ptile([128, 128], bf16, "id_sb")
        qT = [ptile([128, S], f32r, f"qT{m}") for m in range(2)]
        kT = [ptile([128, S], f32r, f"kT{m}") for m in range(2)]
        vext = ptile([128, NT * HPC * 65], f32r, "vext")
        A = [ptile([64, S], f32r, f"A{h}") for h in range(HPC)]

        nc.sync.dma_start(out=wo_sb, in_=dram["wo"])
        nc.sync.dma_start(out=bq_sb, in_=dram["bq"])
        nc.sync.dma_start(out=bk_sb, in_=dram["bk"])
        nc.sync.dma_start(out=tri_sb, in_=dram["tri"])
        nc.sync.dma_start(out=id_sb, in_=dram["ident"])

        vr = vext.rearrange("p (n d) -> p n d", d=65)
        nc.sync.dma_start(
            out=vr[:, :, 64:65],
            in_=dram["vones"].rearrange("p (n o) -> p n o", o=1),
        )

        psum_mm = ctx.enter_context(tc.tile_pool(name="psum_mm", bufs=2, space="PSUM"))
        psum_st = ctx.enter_context(tc.tile_pool(name="psum_st", bufs=3, space="PSUM"))
        psum_o = ctx.enter_context(tc.tile_pool(name="psum_o", bufs=3, space="PSUM"))

        # ---- phase B: projections (x and weight tiles live only here) ----
        with tc.tile_pool(name="xw", bufs=1) as xw:
            xT = [xw.tile([128, S], f32r, name=f"xT{i}") for i in range(KT)]
            wq_sb = xw.tile([128, KT * GD], f32r, name="wq_sb")
            wk_sb = xw.tile([128, KT * GD], f32r, name="wk_sb")
            wv_sb = xw.tile([128, KT * GD], f32r, name="wv_sb")
            for i in range(KT):
                nc.sync.dma_start(out=xT[i], in_=dram["xT"][i * 128 : (i + 1) * 128, :])
            nc.sync.dma_start(out=wq_sb, in_=dram["wq"])
            nc.sync.dma_start(out=wk_sb, in_=dram["wk"])
            nc.sync.dma_start(out=wv_sb, in_=dram["wv"])

            # q/k in transposed layout: [pair-dims 128, seq]
            for w_sb, b_sb, dst in ((wq_sb, bq_sb, qT), (wk_sb, bk_sb, kT)):
                for m in range(2):
                    for sc in range(NSUP):
                        ps = psum_mm.tile([128, 512], f32, tag="mm", name=f"psqk_{m}_{sc}")
                        for kt in range(KT):
                            nc.tensor.matmul(
                                ps,
                                lhsT=_r(w_sb[:, kt * GD + m * 128 : kt * GD + (m + 1) * 128]),
                                rhs=_r(xT[kt][:, sc * 512 : (sc + 1) * 512]),
                                start=(kt == 0),
                                stop=(kt == KT - 1),
                            )
                        nc.vector.tensor_scalar_add(
                            out=dst[m][:, sc * 512 : (sc + 1) * 512],
                            in0=ps,
                            scalar1=b_sb[:, m : m + 1],
                        )

            # v in natural layout [seq, dims], interleaved with ones cols
            for st in range(NT):
                ps = psum_mm.tile([128, 512], f32, tag="mm", name=f"psv_{st}")
                for kt in range(KT):
                    nc.tensor.matmul(
                        ps[:, :GD],
                        lhsT=_r(xT[kt][:, st * 128 : (st + 1) * 128]),
                        rhs=_r(wv_sb[:, kt * GD : (kt + 1) * GD]),
                        start=(kt == 0),
                        stop=(kt == KT - 1),
                    )
                nc.vector.tensor_copy(
                    out=vr[:, st * HPC : (st + 1) * HPC, 0:64],
                    in_=ps[:, :GD].rearrange("p (h d) -> p h d", d=64),
                )

        work = ctx.enter_context(tc.tile_pool(name="work", bufs=4))
        small = ctx.enter_context(tc.tile_pool(name="small", bufs=3))
        ostage = ctx.enter_context(tc.tile_pool(name="ostage", bufs=2))
        dscr = ctx.enter_context(tc.tile_pool(name="dscr", bufs=3, space="DRAM"))

        # ---- phase C/D: attention + output projection, per q supertile ----
        for gq in range(NSUP):
            nk = 4 * gq + 4
            q0 = gq * 512
            for m in range(2):
                po = [
                    psum_o.tile([65, 512], f32, tag="po", name=f"po_{gq}_{m}_{hl}")
                    for hl in range(2)
                ]
                for t in range(nk):
                    c = t - 4 * gq
                    col0 = max(c, 0) * 128
                    for hl in range(2):
                        lo = hl * 64
                        h = 2 * m + hl
                        pst = psum_st.tile(
                            [128, 512], f32, tag="st", name=f"pst_{gq}_{m}_{t}_{hl}"
                        )
                        nc.tensor.matmul(
                            pst[:, col0:],
                            lhsT=_r(kT[m][lo : lo + 64, t * 128 : (t + 1) * 128]),
                            rhs=_r(qT[m][lo : lo + 64, q0 + col0 : q0 + 512]),
                            start=True,
                            stop=(c < 0),
                            skip_group_check=True,
                        )
                        if c >= 0:
                            nc.tensor.matmul(
                                pst[:, c * 128 : (c + 1) * 128],
                                lhsT=tri_sb,
                                rhs=id_sb,
                                start=False,
                                stop=True,
                                skip_group_check=True,
                            )
                        ptt = work.tile(
                            [128, 512], f32r, tag="pt", name=f"pt_{gq}_{m}_{t}_{hl}"
                        )
                        nc.scalar.activation(
                            out=ptt[:, col0:], in_=pst[:, col0:], func=AF.Exp, scale=0.125
                        )
                        nc.tensor.matmul(
                            po[hl][:, col0:],
                            lhsT=_r(vext[:, (t * HPC + h) * 65 : (t * HPC + h) * 65 + 65]),
                            rhs=_r(ptt[:, col0:]),
                            start=(t == 0),
                            stop=(t == nk - 1),
                            skip_group_check=True,
                        )
                for hl in range(2):
                    h = 2 * m + hl
                    rc = small.tile([65, 512], f32, tag="rc", name=f"rc_{gq}_{h}")
                    nc.vector.reciprocal(out=rc[64:65, :], in_=po[hl][64:65, :])
                    scr = dscr.tile([1, 512], f32, tag="scr", name=f"scr_{gq}_{h}")
                    nc.sync.dma_start(out=scr, in_=rc[64:65, :])
                    bc = small.tile([64, 512], f32, tag="bc", name=f"bc_{gq}_{h}")
                    scr_bcast = bass.AP(
                        tensor=scr.tensor, offset=scr.offset, ap=[[0, 64], [1, 512]]
                    )
                    nc.gpsimd.dma_start(out=bc, in_=scr_bcast)
                    nc.vector.tensor_mul(
                        out=A[h][:, q0 : q0 + 512], in0=po[hl][0:64, :], in1=bc
                    )
            for st in range(4 * gq, 4 * gq + 4):
                og = ostage.tile([128, HID], f32, tag="og", name=f"og_{st}")
                for n2 in range(2):
                    ps = psum_mm.tile([128, 512], f32, tag="mm", name=f"pso_{st}_{n2}")
                    for h in range(HPC):
                        nc.tensor.matmul(
                            ps,
                            lhsT=_r(A[h][:, st * 128 : (st + 1) * 128]),
                            rhs=_r(wo_sb[:, h * HID + n2 * 512 : h * HID + (n2 + 1) * 512]),
                            start=(h == 0),
                            stop=(h == HPC - 1),
                        )
                    nc.vector.tensor_copy(out=og[:, n2 * 512 : (n2 + 1) * 512], in_=ps)
                nc.sync.dma_start(out=out[st * 128 : (st + 1) * 128, :], in_=og)


_NC_CACHE = {}


def _get_nc():
    if "nc" in _NC_CACHE:
        return _NC_CACHE["nc"]
    nc = bass.Bass("TRN2", target_bir_lowering=False, debug=False)
    dram = {
        "xT": nc.dram_tensor("xT", [HID, S], f32r, kind="ExternalInput").ap(),
        "wq": nc.dram_tensor("wq", [128, KT * GD], f32r, kind="ExternalInput").ap(),
        "wk": nc.dram_tensor("wk", [128, KT * GD], f32r, kind="ExternalInput").ap(),
        "wv": nc.dram_tensor("wv", [128, KT * GD], f32r, kind="ExternalInput").ap(),
        "wo": nc.dram_tensor("wo", [64, HPC * HID], f32r, kind="ExternalInput").ap(),
        "bq": nc.dram_tensor("bq", [128, 2], f32, kind="ExternalInput").ap(),
        "bk": nc.dram_tensor("bk", [128, 2], f32, kind="ExternalInput").ap(),
        "tri": nc.dram_tensor("tri", [128, 128], bf16, kind="ExternalInput").ap(),
        "ident": nc.dram_tensor("ident", [128, 128], bf16, kind="ExternalInput").ap(),
        "vones": nc.dram_tensor("vones", [128, NT * HPC], f32r, kind="ExternalInput").ap(),
    }
    out = nc.dram_tensor("out", [S, HID], f32, kind="ExternalOutput").ap()
    with tile.TileContext(nc) as tc:
        _build_body(nc, tc, dram, out)
    _split_sem_waits(nc, 1)
    _NC_CACHE["nc"] = nc
    return nc


def kernel(**inputs):
    x = np.ascontiguousarray(np.asarray(inputs["x"], dtype=np.float32))
    Wq = np.asarray(inputs["Wq"], dtype=np.float32)
    Wk = np.asarray(inputs["Wk"], dtype=np.float32)
    Wv = np.asarray(inputs["Wv"], dtype=np.float32)
    Wo = np.asarray(inputs["Wo"], dtype=np.float32)
    bq = np.asarray(inputs["bq"], dtype=np.float32)
    bk = np.asarray(inputs["bk"], dtype=np.float32)
    bv = np.asarray(inputs["bv"], dtype=np.float32)
    bo = np.asarray(inputs["bo"], dtype=np.float32)

    nc = _get_nc()

    tri = np.ascontiguousarray(
        np.triu(np.full((128, 128), -1e9, np.float32), 1).astype(ml_dtypes.bfloat16)
    )
    ident = np.ascontiguousarray(np.eye(128, dtype=np.float32).astype(ml_dtypes.bfloat16))

    in_maps = []
    for core in range(NCORES):
        b, g = divmod(core, HPC)
        sl = slice(g * GD, (g + 1) * GD)
        # wq_dev[p, kt*256+j] = Wq[g*256+j, kt*128+p]
        wq_dev = Wq[sl, :].reshape(GD, KT, 128).transpose(2, 1, 0).reshape(128, KT * GD)
        wk_dev = Wk[sl, :].reshape(GD, KT, 128).transpose(2, 1, 0).reshape(128, KT * GD)
        wv_dev = Wv[sl, :].reshape(GD, KT, 128).transpose(2, 1, 0).reshape(128, KT * GD)
        # wo_dev[p, h*1024+n] = Wo[n, g*256+h*64+p]
        wo_dev = (
            Wo[:, sl].reshape(HID, HPC, 64).transpose(2, 1, 0).reshape(64, HPC * HID)
        )
        in_maps.append(
            {
                "xT": np.ascontiguousarray(x[b].T),
                "wq": np.ascontiguousarray(wq_dev),
                "wk": np.ascontiguousarray(wk_dev),
                "wv": np.ascontiguousarray(wv_dev),
                "wo": np.ascontiguousarray(wo_dev),
                "bq": np.ascontiguousarray(bq[sl].reshape(2, 128).T),
                "bk": np.ascontiguousarray(bk[sl].reshape(2, 128).T),
                "tri": tri,
                "ident": ident,
                "vones": np.ones((128, NT * HPC), dtype=np.float32),
            }
        )

    res = bass_utils.run_bass_kernel_spmd(
        nc, in_maps, core_ids=list(range(NCORES)), trace=TRACE, **TRACE_KW
    )
    if TRACE:
        _NC_CACHE["last_result"] = res

    bias_row = bo + bv @ Wo.T  # softmax rows sum to 1 -> bv passes through
    out = np.empty((B, S, HID), dtype=np.float32)
    for b in range(B):
        acc = res.results[4 * b]["out"].astype(np.float32)
        for g in range(1, HPC):
            acc = acc + res.results[4 * b + g]["out"]
        out[b] = acc + bias_row
    return out


# revision 8
# speedup vs baseline: 1.1992x; 1.1992x over previous
"""Distributed multi-head attention kernel for 8 Trainium2 NeuronCores.

Problem: B=2, S=2048, HIDDEN=1024, 16 heads x 64 dims, causal softmax,
torch-Linear style projections (x @ W.T + b), fp32.

Sharding (tensor parallel over heads x data parallel over batch):
  core c = b*4 + g handles batch b and head-group g (4 heads, 256 dims).
  Each core computes q/k/v projections for its head group, causal
  attention, and a partial output projection A_g @ Wo_g.T -> [S, 1024].
  The host sums the 4 partials per batch (the TP all-reduce) and adds
  the bias terms (bo and bv @ Wo.T, which commute through softmax
  because softmax rows sum to 1).

Device-side layout choices (v2, tuned against HW traces):
  - every matmul uses a full K=128 contraction: K<128 matmuls measure
    ~2x slower (the fast weight-load path needs 128 weight rows), so
    per-head q tiles are zero-padded to 128 partitions and the K=64
    head contractions ride on the zero rows.
  - q is stored per head, zero-padded ([128, seq], data rows at the
    head's position within its pair); k is stored pair-packed
    ([128, seq], heads at rows 0-63/64-127). Scores come out directly
    as S^T[k_pos, q_pos] tiles: lhsT = k pair tile, rhs = padded q.
  - both heads of a pair share one [128, 1024] PSUM score tile
    (2 banks, one 512-wide bank half per head) so the softmax exp is a
    single activation instruction per k-tile (3D access pattern,
    ~30% less ACT overhead than per-head tiles).
  - softmax skips the max-subtraction (scores/8 are O(+-3) here, exp
    cannot overflow); row sums come free from a ones column appended
    to v in the P@V matmul (output row 64 = sum over k of P).
  - causal masking: invalid full 128-col blocks are simply not
    computed (matmul free-dim offset); the diagonal 128x128 block gets
    -1e9 added via one tiny bf16 matmul (strict upper triangular
    constant against identity) before the exp.
  - the P@V matmul runs one k-step behind the score matmul + exp so
    the scalar engine's exp latency stays off the PE critical path.
  - normalization 1/rowsum per 512-wide q chunk: DVE reciprocal on the
    sums row, DMA round-trip through a DRAM scratch to broadcast it
    across 64 partitions, one DVE multiply into A. Odd heads' A data
    is DMA-shifted to partitions 64-127 so the output projection can
    contract pair-packed A tiles with K=128.
  - all large matmuls use float32r (fp32 data, fast PE mode).
"""

import numpy as np
import ml_dtypes

import concourse.bass as bass
import concourse.mybir as mybir
import concourse.tile as tile
from concourse import bass_utils

S = 2048
HID = 1024
D = 64
B = 2
NCORES = 8
HPC = 4            # heads per core
GD = HPC * D       # 256 dims per head group
KT = HID // 128    # 8 hidden k-tiles
NT = S // 128      # 16 seq tiles
NSUP = S // 512    # 4 q supertiles

f32 = mybir.dt.float32
f32r = mybir.dt.float32r
bf16 = mybir.dt.bfloat16
AF = mybir.ActivationFunctionType

TRACE = False
TRACE_KW = {}


def _split_sem_waits(nc, limit=1):
    """walrus in this container accepts only one semaphore wait per
    instruction; hoist excess waits into wait-only EventSemaphore
    instructions inserted just before the offender on the same engine."""
    counter = 0
    for fn in nc.m.functions:
        for blk in fn.blocks:
            out = []
            for inst in blk.instructions:
                si = getattr(inst, "sync_info", None)
                if si is not None:
                    waits = list(si.on_wait)
                    sem_w = [w for w in waits if w.sync_type == "semaphore"]
                    other = [w for w in waits if w.sync_type != "semaphore"]
                    if len(sem_w) > limit:
                        excess, keep = sem_w[:-limit], sem_w[-limit:]
                        for i in range(0, len(excess), limit):
                            counter += 1
                            ev = mybir.InstEventSemaphore(
                                name=f"WSPLIT-{counter}",
                                ins=[],
                                outs=[],
                                sync_info=mybir.SyncInfo(
                                    on_wait=excess[i : i + limit], on_update=[]
                                ),
                            )
                            ev.engine = inst.engine
                            out.append(ev)
                        inst.sync_info = mybir.SyncInfo(
                            on_wait=other + keep, on_update=list(si.on_update)
                        )
                out.append(inst)
            blk.instructions = out


def _build_body(nc, tc, dram, out):
    import contextlib

    with contextlib.ExitStack() as ctx:
        # ---- persistent tiles (bufs=1, one slot per tag) ----
        pers = ctx.enter_context(tc.tile_pool(name="pers", bufs=1))

        def ptile(shape, dtype, name):
            return pers.tile(shape, dtype, name=name, tag=name)

        wo_sb = ptile([128, 2 * HID], f32r, "wo_sb")
        bq_sb = ptile([128, 2], f32, "bq_sb")
        bk_sb = ptile([128, 2], f32, "bk_sb")
        tri_sb = ptile([128, 128], bf16, "tri_sb")
        id_sb = ptile([128, 128], bf16, "id_sb")
        # q per head, zero padded to K=128 (data rows at hl*64)
        qTh = [ptile([128, S], f32r, f"qTh{h}") for h in range(HPC)]
        # k pair-packed (pair m rows 0-63 = head 2m, 64-127 = head 2m+1)
        kT = [ptile([128, S], f32r, f"kT{m}") for m in range(2)]
        vext = ptile([128, NT * HPC * 65], f32r, "vext")
        # A pair-packed for the K=128 output projection
        A = [ptile([128, S], f32r, f"A{m}") for m in range(2)]

        nc.sync.dma_start(out=wo_sb, in_=dram["wo"])
        nc.sync.dma_start(out=bq_sb, in_=dram["bq"])
        nc.sync.dma_start(out=bk_sb, in_=dram["bk"])
        nc.sync.dma_start(out=tri_sb, in_=dram["tri"])
        nc.sync.dma_start(out=id_sb, in_=dram["ident"])
        # zero the padding rows of the per-head q tiles
        for h in range(HPC):
            zrow = 64 if h % 2 == 0 else 0
            nc.sync.dma_start(out=qTh[h][zrow : zrow + 64, :], in_=dram["zpad"])

        vr = vext.rearrange("p (n d) -> p n d", d=65)
        nc.sync.dma_start(
            out=vr[:, :, 64:65],
            in_=dram["vones"].rearrange("p (n o) -> p n o", o=1),
        )

        psum_mm = ctx.enter_context(tc.tile_pool(name="psum_mm", bufs=2, space="PSUM"))
        psum_st = ctx.enter_context(tc.tile_pool(name="psum_st", bufs=2, space="PSUM"))
        psum_o = ctx.enter_context(tc.tile_pool(name="psum_o", bufs=2, space="PSUM"))

        small = ctx.enter_context(tc.tile_pool(name="small", bufs=3))

        # warm the ACT exp table while DMAs run
        warm = small.tile([128, 2], f32, tag="warm", name="warm")
        nc.scalar.activation(out=warm, in_=bq_sb, func=AF.Exp, scale=0.0)

        # ---- phase B: projections (x and weight tiles live only here) ----
        with tc.tile_pool(name="xw", bufs=1) as xw, tc.tile_pool(
            name="wrot", bufs=2
        ) as wrot:
            xT = [xw.tile([128, S], f32r, name=f"xT{i}") for i in range(KT)]
            for i in range(KT):
                nc.sync.dma_start(out=xT[i], in_=dram["xT"][i * 128 : (i + 1) * 128, :])

            def proj_qk(which, b_sb, pair_dst, head_dst):
                w_sb = wrot.tile([128, KT * GD], f32r, tag="w", name=f"w_{which}")
                nc.sync.dma_start(out=w_sb, in_=dram[which])
                for m in range(2):
                    for sc in range(NSUP):
                        ps = psum_mm.tile(
                            [128, 512], f32, tag="mm", name=f"ps{which}_{m}_{sc}"
                        )
                        for kt in range(KT):
                            nc.tensor.matmul(
                                ps,
                                lhsT=w_sb[:, kt * GD + m * 128 : kt * GD + (m + 1) * 128],
                                rhs=xT[kt][:, sc * 512 : (sc + 1) * 512],
                                start=(kt == 0),
                                stop=(kt == KT - 1),
                            )
                        for hl in range(2):
                            lo = hl * 64
                            dst = (
                                pair_dst[m] if head_dst is None else head_dst[2 * m + hl]
                            )
                            nc.vector.tensor_scalar_add(
                                out=dst[lo : lo + 64, sc * 512 : (sc + 1) * 512],
                                in0=ps[lo : lo + 64, :],
                                scalar1=b_sb[lo : lo + 64, m : m + 1],
                            )

            proj_qk("wq", bq_sb, None, qTh)
            proj_qk("wk", bk_sb, kT, None)

            # v in natural layout [seq, dims], interleaved with ones cols
            wv_sb = wrot.tile([128, KT * GD], f32r, tag="w", name="w_wv")
            nc.sync.dma_start(out=wv_sb, in_=dram["wv"])
            for st in range(NT):
                ps = psum_mm.tile([128, 512], f32, tag="mm", name=f"psv_{st}")
                for kt in range(KT):
                    nc.tensor.matmul(
                        ps[:, :GD],
                        lhsT=xT[kt][:, st * 128 : (st + 1) * 128],
                        rhs=wv_sb[:, kt * GD : (kt + 1) * GD],
                        start=(kt == 0),
                        stop=(kt == KT - 1),
                    )
                nc.vector.tensor_copy(
                    out=vr[:, st * HPC : (st + 1) * HPC, 0:64],
                    in_=ps[:, :GD].rearrange("p (h d) -> p h d", d=64),
                )

        work = ctx.enter_context(tc.tile_pool(name="work", bufs=4))
        ostage = ctx.enter_context(tc.tile_pool(name="ostage", bufs=2))
        dscr = ctx.enter_context(tc.tile_pool(name="dscr", bufs=3, space="DRAM"))

        # ---- phase C/D: attention + output projection, per q supertile ----
        for gq in range(NSUP):
            nk = 4 * gq + 4
            q0 = gq * 512
            for m in range(2):
                po = [
                    psum_o.tile([65, 512], f32, tag="po", name=f"po_{gq}_{m}_{hl}")
                    for hl in range(2)
                ]
                # 1-deep software pipeline: PV(t) runs after ST/exp(t+1) is
                # issued so PE never sits behind the ACT exp.
                pts = {}
                for t in range(nk):
                    c = t - 4 * gq
                    col0 = max(c, 0) * 128
                    pst = psum_st.tile(
                        [128, 1024], f32, tag="st", name=f"pst_{gq}_{m}_{t}"
                    )
                    for hl in range(2):
                        h0 = hl * 512
                        nc.tensor.matmul(
                            pst[:, h0 + col0 : h0 + 512],
                            lhsT=kT[m][:, t * 128 : (t + 1) * 128],
                            rhs=qTh[2 * m + hl][:, q0 + col0 : q0 + 512],
                            start=True,
                            stop=(c < 0),
                            skip_group_check=True,
                        )
                        if c >= 0:
                            nc.tensor.matmul(
                                pst[:, h0 + c * 128 : h0 + (c + 1) * 128],
                                lhsT=tri_sb,
                                rhs=id_sb,
                                start=False,
                                stop=True,
                                skip_group_check=True,
                            )
                    ptt = work.tile([128, 1024], f32r, tag="pt", name=f"pt_{gq}_{m}_{t}")
                    nc.scalar.activation(
                        out=ptt.rearrange("p (h n) -> p h n", h=2)[:, :, col0:],
                        in_=pst.rearrange("p (h n) -> p h n", h=2)[:, :, col0:],
                        func=AF.Exp,
                        scale=0.125,
                    )
                    pts[t] = ptt

                    def pv(tt):
                        cc = max(tt - 4 * gq, 0) * 128
                        for hl2 in range(2):
                            h = 2 * m + hl2
                            nc.tensor.matmul(
                                po[hl2][:, cc:],
                                lhsT=vext[:, (tt * HPC + h) * 65 : (tt * HPC + h) * 65 + 65],
                                rhs=pts[tt][:, hl2 * 512 + cc : (hl2 + 1) * 512],
                                start=(tt == 0),
                                stop=(tt == nk - 1),
                                skip_group_check=True,
                            )

                    if t > 0:
                        pv(t - 1)
                pv(nk - 1)
                for hl in range(2):
                    h = 2 * m + hl
                    rc = small.tile([65, 512], f32, tag="rc", name=f"rc_{gq}_{h}")
                    nc.vector.reciprocal(out=rc[64:65, :], in_=po[hl][64:65, :])
                    scr = dscr.tile([1, 512], f32, tag="scr", name=f"scr_{gq}_{h}")
                    nc.sync.dma_start(out=scr, in_=rc[64:65, :])
                    bc = small.tile([64, 512], f32, tag="bc", name=f"bc_{gq}_{h}")
                    scr_bcast = bass.AP(
                        tensor=scr.tensor, offset=scr.offset, ap=[[0, 64], [1, 512]]
                    )
                    nc.gpsimd.dma_start(out=bc, in_=scr_bcast)
                    if hl == 0:
                        nc.vector.tensor_mul(
                            out=A[m][0:64, q0 : q0 + 512], in0=po[hl][0:64, :], in1=bc
                        )
                    else:
                        atmp = small.tile([64, 512], f32r, tag="atmp", name=f"atmp_{gq}_{h}")
                        nc.vector.tensor_mul(out=atmp, in0=po[hl][0:64, :], in1=bc)
                        nc.sync.dma_start(
                            out=A[m][64:128, q0 : q0 + 512], in_=atmp
                        )
            for st in range(4 * gq, 4 * gq + 4):
                og = ostage.tile([128, HID], f32, tag="og", name=f"og_{st}")
                for n2 in range(2):
                    ps = psum_mm.tile([128, 512], f32, tag="mm", name=f"pso_{st}_{n2}")
                    for m in range(2):
                        nc.tensor.matmul(
                            ps,
                            lhsT=A[m][:, st * 128 : (st + 1) * 128],
                            rhs=wo_sb[:, m * HID + n2 * 512 : m * HID + (n2 + 1) * 512],
                            start=(m == 0),
                            stop=(m == 1),
                        )
                    nc.vector.tensor_copy(out=og[:, n2 * 512 : (n2 + 1) * 512], in_=ps)
                nc.sync.dma_start(out=out[st * 128 : (st + 1) * 128, :], in_=og)


_NC_CACHE = {}


def _get_nc():
    if "nc" in _NC_CACHE:
        return _NC_CACHE["nc"]
    nc = bass.Bass("TRN2", target_bir_lowering=False, debug=False)
    dram = {
        "xT": nc.dram_tensor("xT", [HID, S], f32r, kind="ExternalInput").ap(),
        "wq": nc.dram_tensor("wq", [128, KT * GD], f32r, kind="ExternalInput").ap(),
        "wk": nc.dram_tensor("wk", [128, KT * GD], f32r, kind="ExternalInput").ap(),
        "wv": nc.dram_tensor("wv", [128, KT * GD], f32r, kind="ExternalInput").ap(),
        "wo": nc.dram_tensor("wo", [128, 2 * HID], f32r, kind="ExternalInput").ap(),
        "bq": nc.dram_tensor("bq", [128, 2], f32, kind="ExternalInput").ap(),
        "bk": nc.dram_tensor("bk", [128, 2], f32, kind="ExternalInput").ap(),
        "tri": nc.dram_tensor("tri", [128, 128], bf16, kind="ExternalInput").ap(),
        "ident": nc.dram_tensor("ident", [128, 128], bf16, kind="ExternalInput").ap(),
        "vones": nc.dram_tensor("vones", [128, NT * HPC], f32r, kind="ExternalInput").ap(),
        "zpad": nc.dram_tensor("zpad", [64, S], f32r, kind="ExternalInput").ap(),
    }
    out = nc.dram_tensor("out", [S, HID], f32, kind="ExternalOutput").ap()
    with tile.TileContext(nc) as tc:
        _build_body(nc, tc, dram, out)
    _split_sem_waits(nc, 1)
    _NC_CACHE["nc"] = nc
    return nc


def kernel(**inputs):
    x = np.ascontiguousarray(np.asarray(inputs["x"], dtype=np.float32))
    Wq = np.asarray(inputs["Wq"], dtype=np.float32)
    Wk = np.asarray(inputs["Wk"], dtype=np.float32)
    Wv = np.asarray(inputs["Wv"], dtype=np.float32)
    Wo = np.asarray(inputs["Wo"], dtype=np.float32)
    bq = np.asarray(inputs["bq"], dtype=np.float32)
    bk = np.asarray(inputs["bk"], dtype=np.float32)
    bv = np.asarray(inputs["bv"], dtype=np.float32)
    bo = np.asarray(inputs["bo"], dtype=np.float32)

    nc = _get_nc()

    tri = np.ascontiguousarray(
        np.triu(np.full((128, 128), -1e9, np.float32), 1).astype(ml_dtypes.bfloat16)
    )
    ident = np.ascontiguousarray(np.eye(128, dtype=np.float32).astype(ml_dtypes.bfloat16))
    vones = np.ones((128, NT * HPC), dtype=np.float32)
    zpad = np.zeros((64, S), dtype=np.float32)

    in_maps = []
    for core in range(NCORES):
        b, g = divmod(core, HPC)
        sl = slice(g * GD, (g + 1) * GD)
        # wq_dev[p, kt*256+j] = Wq[g*256+j, kt*128+p]
        wq_dev = Wq[sl, :].reshape(GD, KT, 128).transpose(2, 1, 0).reshape(128, KT * GD)
        wk_dev = Wk[sl, :].reshape(GD, KT, 128).transpose(2, 1, 0).reshape(128, KT * GD)
        wv_dev = Wv[sl, :].reshape(GD, KT, 128).transpose(2, 1, 0).reshape(128, KT * GD)
        # wo_dev[p, m*1024+n] = Wo[n, g*256+m*128+p]
        wo_dev = Wo[:, sl].reshape(HID, 2, 128).transpose(2, 1, 0).reshape(128, 2 * HID)
        in_maps.append(
            {
                "xT": np.ascontiguousarray(x[b].T),
                "wq": np.ascontiguousarray(wq_dev),
                "wk": np.ascontiguousarray(wk_dev),
                "wv": np.ascontiguousarray(wv_dev),
                "wo": np.ascontiguousarray(wo_dev),
                "bq": np.ascontiguousarray(bq[sl].reshape(2, 128).T),
                "bk": np.ascontiguousarray(bk[sl].reshape(2, 128).T),
                "tri": tri,
                "ident": ident,
                "vones": vones,
                "zpad": zpad,
            }
        )

    res = bass_utils.run_bass_kernel_spmd(
        nc, in_maps, core_ids=list(range(NCORES)), trace=TRACE, **TRACE_KW
    )
    if TRACE:
        _NC_CACHE["last_result"] = res

    bias_row = bo + bv @ Wo.T  # softmax rows sum to 1 -> bv passes through
    out = np.empty((B, S, HID), dtype=np.float32)
    for b in range(B):
        acc = res.results[4 * b]["out"].astype(np.float32)
        for g in range(1, HPC):
            acc = acc + res.results[4 * b + g]["out"]
        out[b] = acc + bias_row
    return out


# revision 12
# speedup vs baseline: 1.4719x; 1.2274x over previous
"""Distributed multi-head attention kernel for 8 Trainium2 NeuronCores.

Problem: B=2, S=2048, HIDDEN=1024, 16 heads x 64 dims, causal softmax,
torch-Linear style projections (x @ W.T + b), fp32.

Sharding (tensor parallel over heads x data parallel over batch):
  core c = b*4 + g handles batch b and head-group g (4 heads, 256 dims).
  Each core computes q/k/v projections for its head group, causal
  attention, and a partial output projection A_g @ Wo_g.T -> [S, 1024].
  The host sums the 4 partials per batch (the TP all-reduce) and adds
  the bias terms (bo and bv @ Wo.T, which commute through softmax
  because softmax rows sum to 1).

Device-side layout choices (v2, tuned against HW traces):
  - every matmul uses a full K=128 contraction: K<128 matmuls measure
    ~2x slower (the fast weight-load path needs 128 weight rows), so
    per-head q tiles are zero-padded to 128 partitions and the K=64
    head contractions ride on the zero rows.
  - q is stored per head, zero-padded ([128, seq], data rows at the
    head's position within its pair); k is stored pair-packed
    ([128, seq], heads at rows 0-63/64-127). Scores come out directly
    as S^T[k_pos, q_pos] tiles: lhsT = k pair tile, rhs = padded q.
  - both heads of a pair share one [128, 1024] PSUM score tile
    (2 banks, one 512-wide bank half per head) so the softmax exp is a
    single activation instruction per k-tile (3D access pattern,
    ~30% less ACT overhead than per-head tiles).
  - softmax skips the max-subtraction (scores/8 are O(+-3) here, exp
    cannot overflow); row sums come free from a ones column appended
    to v in the P@V matmul (output row 64 = sum over k of P).
  - causal masking: invalid full 128-col blocks are simply not
    computed (matmul free-dim offset); the diagonal 128x128 block gets
    -1e9 added via one tiny bf16 matmul (strict upper triangular
    constant against identity) before the exp.
  - the P@V matmul runs one k-step behind the score matmul + exp so
    the scalar engine's exp latency stays off the PE critical path.
  - normalization 1/rowsum per 512-wide q chunk: DVE reciprocal on the
    sums row, DMA round-trip through a DRAM scratch to broadcast it
    across 64 partitions, one DVE multiply into A. Odd heads' A data
    is DMA-shifted to partitions 64-127 so the output projection can
    contract pair-packed A tiles with K=128.
  - all large matmuls use float32r (fp32 data, fast PE mode).
"""

import numpy as np
import ml_dtypes

import concourse.bass as bass
import concourse.mybir as mybir
import concourse.tile as tile
from concourse import bass_utils

S = 2048
HID = 1024
D = 64
B = 2
NCORES = 8
HPC = 4            # heads per core
GD = HPC * D       # 256 dims per head group
KT = HID // 128    # 8 hidden k-tiles
NT = S // 128      # 16 seq tiles
NSUP = S // 512    # 4 q supertiles

f32 = mybir.dt.float32
f32r = mybir.dt.float32r
bf16 = mybir.dt.bfloat16
AF = mybir.ActivationFunctionType

TRACE = False
TRACE_KW = {}


def _split_sem_waits(nc, limit=1):
    """walrus in this container accepts only one semaphore wait per
    instruction; hoist excess waits into wait-only EventSemaphore
    instructions inserted just before the offender on the same engine."""
    counter = 0
    for fn in nc.m.functions:
        for blk in fn.blocks:
            out = []
            for inst in blk.instructions:
                si = getattr(inst, "sync_info", None)
                if si is not None:
                    waits = list(si.on_wait)
                    sem_w = [w for w in waits if w.sync_type == "semaphore"]
                    other = [w for w in waits if w.sync_type != "semaphore"]
                    if len(sem_w) > limit:
                        excess, keep = sem_w[:-limit], sem_w[-limit:]
                        for i in range(0, len(excess), limit):
                            counter += 1
                            ev = mybir.InstEventSemaphore(
                                name=f"WSPLIT-{counter}",
                                ins=[],
                                outs=[],
                                sync_info=mybir.SyncInfo(
                                    on_wait=excess[i : i + limit], on_update=[]
                                ),
                            )
                            ev.engine = inst.engine
                            out.append(ev)
                        inst.sync_info = mybir.SyncInfo(
                            on_wait=other + keep, on_update=list(si.on_update)
                        )
                out.append(inst)
            blk.instructions = out


def _build_body(nc, tc, dram, out):
    import contextlib

    with contextlib.ExitStack() as ctx:
        # ---- persistent tiles (bufs=1, one slot per tag) ----
        pers = ctx.enter_context(tc.tile_pool(name="pers", bufs=1))

        def ptile(shape, dtype, name):
            return pers.tile(shape, dtype, name=name, tag=name)

        wo_sb = ptile([128, 2 * HID], f32r, "wo_sb")
        bq_sb = ptile([128, 2], f32, "bq_sb")
        bk_sb = ptile([128, 2], f32, "bk_sb")
        tri_sb = ptile([128, 128], bf16, "tri_sb")
        id_sb = ptile([128, 128], bf16, "id_sb")
        # q per head, zero padded to K=128 (data rows at hl*64)
        qTh = [ptile([128, S], f32r, f"qTh{h}") for h in range(HPC)]
        # k pair-packed (pair m rows 0-63 = head 2m, 64-127 = head 2m+1)
        kT = [ptile([128, S], f32r, f"kT{m}") for m in range(2)]
        vext = ptile([128, NT * HPC * 65], f32r, "vext")
        # A pair-packed for the K=128 output projection
        A = [ptile([128, S], f32r, f"A{m}") for m in range(2)]

        nc.sync.dma_start(out=bq_sb, in_=dram["bq"])
        nc.sync.dma_start(out=bk_sb, in_=dram["bk"])
        vr = vext.rearrange("p (n d) -> p n d", d=65)

        psum_mm = ctx.enter_context(tc.tile_pool(name="psum_mm", bufs=2, space="PSUM"))
        psum_st = ctx.enter_context(tc.tile_pool(name="psum_st", bufs=2, space="PSUM"))
        psum_o = ctx.enter_context(tc.tile_pool(name="psum_o", bufs=2, space="PSUM"))

        pre = ctx.enter_context(tc.tile_pool(name="pre", bufs=1))

        # warm the ACT exp table while DMAs run
        warm = pre.tile([128, 2], f32, tag="warm", name="warm")
        nc.scalar.activation(out=warm, in_=bq_sb, func=AF.Exp, scale=0.0)

        # ---- phase B: projections (x and weight tiles live only here) ----
        with tc.tile_pool(name="xw", bufs=1) as xw, tc.tile_pool(
            name="wrot", bufs=2
        ) as wrot:
            xT = [xw.tile([128, S], f32r, name=f"xT{i}") for i in range(KT)]
            dma_engines = [nc.sync, nc.scalar]
            for i in range(KT):
                dma_engines[i % 2].dma_start(
                    out=xT[i], in_=dram["xT"][i * 128 : (i + 1) * 128, :]
                )

            def proj_qk(which, b_sb, pair_dst, head_dst):
                w_sb = wrot.tile([128, KT * GD], f32r, tag="w", name=f"w_{which}")
                (nc.sync if which == "wq" else nc.gpsimd).dma_start(
                    out=w_sb, in_=dram[which]
                )
                for m in range(2):
                    for sc in range(NSUP):
                        ps = psum_mm.tile(
                            [128, 512], f32, tag="mm", name=f"ps{which}_{m}_{sc}"
                        )
                        for kt in range(KT):
                            nc.tensor.matmul(
                                ps,
                                lhsT=w_sb[:, kt * GD + m * 128 : kt * GD + (m + 1) * 128],
                                rhs=xT[kt][:, sc * 512 : (sc + 1) * 512],
                                start=(kt == 0),
                                stop=(kt == KT - 1),
                            )
                        for hl in range(2):
                            lo = hl * 64
                            dst = (
                                pair_dst[m] if head_dst is None else head_dst[2 * m + hl]
                            )
                            nc.vector.tensor_scalar_add(
                                out=dst[lo : lo + 64, sc * 512 : (sc + 1) * 512],
                                in0=ps[lo : lo + 64, :],
                                scalar1=b_sb[lo : lo + 64, m : m + 1],
                            )

            proj_qk("wq", bq_sb, None, qTh)
            proj_qk("wk", bk_sb, kT, None)

            # v in natural layout [seq, dims], interleaved with ones cols
            wv_sb = wrot.tile([128, KT * GD], f32r, tag="w", name="w_wv")
            nc.gpsimd.dma_start(out=wv_sb, in_=dram["wv"])
            for st in range(NT):
                ps = psum_mm.tile([128, 512], f32, tag="mm", name=f"psv_{st}")
                for kt in range(KT):
                    nc.tensor.matmul(
                        ps[:, :GD],
                        lhsT=xT[kt][:, st * 128 : (st + 1) * 128],
                        rhs=wv_sb[:, kt * GD : (kt + 1) * GD],
                        start=(kt == 0),
                        stop=(kt == KT - 1),
                    )
                nc.vector.tensor_copy(
                    out=vr[:, st * HPC : (st + 1) * HPC, 0:64],
                    in_=ps[:, :GD].rearrange("p (h d) -> p h d", d=64),
                )

        nc.scalar.dma_start(out=tri_sb, in_=dram["tri"])
        nc.scalar.dma_start(out=id_sb, in_=dram["ident"])
        # zero the padding rows of the per-head q tiles
        for h in range(HPC):
            zrow = 64 if h % 2 == 0 else 0
            nc.gpsimd.dma_start(out=qTh[h][zrow : zrow + 64, :], in_=dram["zpad"])
        nc.sync.dma_start(out=wo_sb, in_=dram["wo"])
        # ones columns for the P@V row sums; slow strided descriptor gen,
        # keep it off the main sync queue
        nc.gpsimd.dma_start(
            out=vr[:, :, 64:65],
            in_=dram["vones"].rearrange("p (n o) -> p n o", o=1),
        )

        work = ctx.enter_context(tc.tile_pool(name="work", bufs=4))
        small = ctx.enter_context(tc.tile_pool(name="small", bufs=2))
        ostage = ctx.enter_context(tc.tile_pool(name="ostage", bufs=2))
        dscr = ctx.enter_context(tc.tile_pool(name="dscr", bufs=3, space="DRAM"))

        # ---- phase C/D: attention + output projection, per q supertile ----
        for gq in range(NSUP):
            nk = 4 * gq + 4
            q0 = gq * 512
            scr4 = dscr.tile([HPC, 512], f32, tag="scr4", name=f"scr4_{gq}")
            araw = [
                work.tile([64, 512], f32, tag=f"araw{h}", name=f"araw_{gq}_{h}", bufs=2)
                for h in range(HPC)
            ]
            for m in range(2):
                po = [
                    psum_o.tile([65, 512], f32, tag="po", name=f"po_{gq}_{m}_{hl}")
                    for hl in range(2)
                ]
                # 1-deep software pipeline: PV(t) runs after ST/exp(t+1) is
                # issued so PE never sits behind the ACT exp.
                pts = {}
                for t in range(nk):
                    c = t - 4 * gq
                    col0 = max(c, 0) * 128
                    pst = psum_st.tile(
                        [128, 1024], f32, tag="st", name=f"pst_{gq}_{m}_{t}"
                    )
                    for hl in range(2):
                        h0 = hl * 512
                        nc.tensor.matmul(
                            pst[:, h0 + col0 : h0 + 512],
                            lhsT=kT[m][:, t * 128 : (t + 1) * 128],
                            rhs=qTh[2 * m + hl][:, q0 + col0 : q0 + 512],
                            start=True,
                            stop=(c < 0),
                            skip_group_check=True,
                        )
                        if c >= 0:
                            nc.tensor.matmul(
                                pst[:, h0 + c * 128 : h0 + (c + 1) * 128],
                                lhsT=tri_sb,
                                rhs=id_sb,
                                start=False,
                                stop=True,
                                skip_group_check=True,
                            )
                    ptt = work.tile([128, 1024], f32r, tag="pt", name=f"pt_{gq}_{m}_{t}")
                    nc.scalar.activation(
                        out=ptt.rearrange("p (h n) -> p h n", h=2)[:, :, col0:],
                        in_=pst.rearrange("p (h n) -> p h n", h=2)[:, :, col0:],
                        func=AF.Exp,
                        scale=0.125,
                    )
                    pts[t] = ptt

                    def pv(tt):
                        cc = max(tt - 4 * gq, 0) * 128
                        for hl2 in range(2):
                            h = 2 * m + hl2
                            nc.tensor.matmul(
                                po[hl2][:, cc:],
                                lhsT=vext[:, (tt * HPC + h) * 65 : (tt * HPC + h) * 65 + 65],
                                rhs=pts[tt][:, hl2 * 512 + cc : (hl2 + 1) * 512],
                                start=(tt == 0),
                                stop=(tt == nk - 1),
                                skip_group_check=True,
                            )

                    if t > 1:
                        pv(t - 2)
                pv(nk - 2)
                pv(nk - 1)
                # drain PSUM fast: raw attention rows to SBUF (DVE), sum
                # rows to SBUF (ACT) then to the per-super DRAM gather
                for hl in range(2):
                    h = 2 * m + hl
                    ar = araw[h]
                    nc.vector.tensor_copy(out=ar, in_=po[hl][0:64, :])
                    srow = small.tile([65, 512], f32, tag="srow", name=f"srow_{gq}_{h}")
                    nc.scalar.copy(out=srow[64:65, :], in_=po[hl][64:65, :])
                    nc.sync.dma_start(out=scr4[2 * m + hl : 2 * m + hl + 1, :], in_=srow[64:65, :])
            rc = small.tile([HPC, 512], f32, tag="rc", name=f"rc_{gq}")
            nc.sync.dma_start(out=rc, in_=scr4)
            rr = small.tile([HPC, 512], f32, tag="rr", name=f"rr_{gq}")
            nc.vector.reciprocal(out=rr, in_=rc)
            scr4r = dscr.tile([HPC, 512], f32, tag="scr4r", name=f"scr4r_{gq}")
            nc.sync.dma_start(out=scr4r, in_=rr)
            for h in range(HPC):
                m, hl = divmod(h, 2)
                bc = small.tile([64, 512], f32, tag="bc", name=f"bc_{gq}_{h}")
                bcast_ap = bass.AP(
                    tensor=scr4r.tensor, offset=scr4r.offset + h * 512,
                    ap=[[0, 64], [1, 512]],
                )
                nc.gpsimd.dma_start(out=bc, in_=bcast_ap)
                if hl == 0:
                    nc.vector.tensor_mul(
                        out=A[m][0:64, q0 : q0 + 512], in0=araw[h], in1=bc
                    )
                else:
                    atmp = small.tile([64, 512], f32r, tag="atmp", name=f"atmp_{gq}_{h}")
                    nc.vector.tensor_mul(out=atmp, in0=araw[h], in1=bc)
                    nc.sync.dma_start(out=A[m][64:128, q0 : q0 + 512], in_=atmp)
            for st in range(4 * gq, 4 * gq + 4):
                og = ostage.tile([128, HID], f32, tag="og", name=f"og_{st}")
                for n2 in range(2):
                    ps = psum_mm.tile([128, 512], f32, tag="mm", name=f"pso_{st}_{n2}")
                    for m in range(2):
                        nc.tensor.matmul(
                            ps,
                            lhsT=A[m][:, st * 128 : (st + 1) * 128],
                            rhs=wo_sb[:, m * HID + n2 * 512 : m * HID + (n2 + 1) * 512],
                            start=(m == 0),
                            stop=(m == 1),
                        )
                    nc.vector.tensor_copy(out=og[:, n2 * 512 : (n2 + 1) * 512], in_=ps)
                nc.sync.dma_start(out=out[st * 128 : (st + 1) * 128, :], in_=og)


_NC_CACHE = {}


def _get_nc():
    if "nc" in _NC_CACHE:
        return _NC_CACHE["nc"]
    nc = bass.Bass("TRN2", target_bir_lowering=False, debug=False)
    dram = {
        "xT": nc.dram_tensor("xT", [HID, S], f32r, kind="ExternalInput").ap(),
        "wq": nc.dram_tensor("wq", [128, KT * GD], f32r, kind="ExternalInput").ap(),
        "wk": nc.dram_tensor("wk", [128, KT * GD], f32r, kind="ExternalInput").ap(),
        "wv": nc.dram_tensor("wv", [128, KT * GD], f32r, kind="ExternalInput").ap(),
        "wo": nc.dram_tensor("wo", [128, 2 * HID], f32r, kind="ExternalInput").ap(),
        "bq": nc.dram_tensor("bq", [128, 2], f32, kind="ExternalInput").ap(),
        "bk": nc.dram_tensor("bk", [128, 2], f32, kind="ExternalInput").ap(),
        "tri": nc.dram_tensor("tri", [128, 128], bf16, kind="ExternalInput").ap(),
        "ident": nc.dram_tensor("ident", [128, 128], bf16, kind="ExternalInput").ap(),
        "vones": nc.dram_tensor("vones", [128, NT * HPC], f32r, kind="ExternalInput").ap(),
        "zpad": nc.dram_tensor("zpad", [64, S], f32r, kind="ExternalInput").ap(),
    }
    out = nc.dram_tensor("out", [S, HID], f32, kind="ExternalOutput").ap()
    with tile.TileContext(nc) as tc:
        _build_body(nc, tc, dram, out)
    _split_sem_waits(nc, 1)
    _NC_CACHE["nc"] = nc
    return nc


def kernel(**inputs):
    x = np.ascontiguousarray(np.asarray(inputs["x"], dtype=np.float32))
    Wq = np.asarray(inputs["Wq"], dtype=np.float32)
    Wk = np.asarray(inputs["Wk"], dtype=np.float32)
    Wv = np.asarray(inputs["Wv"], dtype=np.float32)
    Wo = np.asarray(inputs["Wo"], dtype=np.float32)
    bq = np.asarray(inputs["bq"], dtype=np.float32)
    bk = np.asarray(inputs["bk"], dtype=np.float32)
    bv = np.asarray(inputs["bv"], dtype=np.float32)
    bo = np.asarray(inputs["bo"], dtype=np.float32)

    nc = _get_nc()

    tri = np.ascontiguousarray(
        np.triu(np.full((128, 128), -1e9, np.float32), 1).astype(ml_dtypes.bfloat16)
    )
    ident = np.ascontiguousarray(np.eye(128, dtype=np.float32).astype(ml_dtypes.bfloat16))
    vones = np.ones((128, NT * HPC), dtype=np.float32)
    zpad = np.zeros((64, S), dtype=np.float32)

    in_maps = []
    for core in range(NCORES):
        b, g = divmod(core, HPC)
        sl = slice(g * GD, (g + 1) * GD)
        # wq_dev[p, kt*256+j] = Wq[g*256+j, kt*128+p]
        wq_dev = Wq[sl, :].reshape(GD, KT, 128).transpose(2, 1, 0).reshape(128, KT * GD)
        wk_dev = Wk[sl, :].reshape(GD, KT, 128).transpose(2, 1, 0).reshape(128, KT * GD)
        wv_dev = Wv[sl, :].reshape(GD, KT, 128).transpose(2, 1, 0).reshape(128, KT * GD)
        # wo_dev[p, m*1024+n] = Wo[n, g*256+m*128+p]
        wo_dev = Wo[:, sl].reshape(HID, 2, 128).transpose(2, 1, 0).reshape(128, 2 * HID)
        in_maps.append(
            {
                "xT": np.ascontiguousarray(x[b].T),
                "wq": np.ascontiguousarray(wq_dev),
                "wk": np.ascontiguousarray(wk_dev),
                "wv": np.ascontiguousarray(wv_dev),
                "wo": np.ascontiguousarray(wo_dev),
                "bq": np.ascontiguousarray(bq[sl].reshape(2, 128).T),
                "bk": np.ascontiguousarray(bk[sl].reshape(2, 128).T),
                "tri": tri,
                "ident": ident,
                "vones": vones,
                "zpad": zpad,
            }
        )

    res = bass_utils.run_bass_kernel_spmd(
        nc, in_maps, core_ids=list(range(NCORES)), trace=TRACE, **TRACE_KW
    )
    if TRACE:
        _NC_CACHE["last_result"] = res

    bias_row = bo + bv @ Wo.T  # softmax rows sum to 1 -> bv passes through
    out = np.empty((B, S, HID), dtype=np.float32)
    for b in range(B):
        acc = res.results[4 * b]["out"].astype(np.float32)
        for g in range(1, HPC):
            acc = acc + res.results[4 * b + g]["out"]
        out[b] = acc + bias_row
    return out
